# revision 32
# baseline (speedup 1.0000x reference)
"""v2.4 reconstruction: best-measured config (36793 ns).

q1: obs0, pol, act, obs1, obs2, obs3A, obs3B, xw0-3.  q10: cst, cstb.
P/A entirely on gpsimd; trees/lr/combines on DVE; chains on PE/ACT;
prefix0-2 gpsimd, prefix3 DVE from the pw PSUM columns.
"""

import numpy as np
import ml_dtypes
from contextlib import ExitStack

import concourse.bass as bass
import concourse.bacc as bacc
import concourse.tile as tile
from concourse import mybir
from concourse.bass_utils import run_bass_kernel_spmd

B, N, A = 4096, 16, 8
D_IN, H1, DP, DZ = 128, 64, 64, 64
NCORES = 8
BC = B // NCORES
RC = BC * N
G = 4
GE = BC // G

F32 = mybir.dt.float32
BF16 = mybir.dt.bfloat16
ALU = mybir.AluOpType
AFT = mybir.ActivationFunctionType


def _dep(later, earlier):
    if later is not None and earlier is not None:
        tile.add_dep_helper(later.ins, earlier.ins, sync=False,
                            reason="engine stream order")


def _build():
    nc = bacc.Bacc("TRN2", target_bir_lowering=False, debug=False)

    obs = nc.dram_tensor("obs", [RC, D_IN], F32, kind="ExternalInput")
    pol = nc.dram_tensor("pol", [RC, A], F32, kind="ExternalInput")
    act = nc.dram_tensor("act", [RC, A], F32, kind="ExternalInput")
    cst = nc.dram_tensor("cst", [128, 10], F32, kind="ExternalInput")
    cstb = nc.dram_tensor("cstb", [128, 452], BF16, kind="ExternalInput")
    xw = nc.dram_tensor("xw", [RC, 2 * N], F32, kind="ExternalOutput")

    with ExitStack() as ctx:
        tc = ctx.enter_context(tile.TileContext(nc))
        consts = ctx.enter_context(tc.tile_pool(name="consts", bufs=1))
        obsp = ctx.enter_context(tc.tile_pool(name="obsp", bufs=1))
        trp = ctx.enter_context(tc.tile_pool(name="trp", bufs=2))
        pap = ctx.enter_context(tc.tile_pool(name="pap", bufs=1))
        gsp = ctx.enter_context(tc.tile_pool(name="gsp", bufs=1))
        smp = ctx.enter_context(tc.tile_pool(name="smp", bufs=1))
        pmtp = ctx.enter_context(tc.tile_pool(name="pmtp", bufs=2, space="PSUM"))
        php = ctx.enter_context(tc.tile_pool(name="php", bufs=2, space="PSUM"))
        pacp = ctx.enter_context(tc.tile_pool(name="pacp", bufs=2, space="PSUM"))
        pwp = ctx.enter_context(tc.tile_pool(name="pwp", bufs=2, space="PSUM"))

        obs_v = obs.ap().rearrange("(p s n) f -> s p (n f)", p=128, n=16)
        obs_t = []
        t = obsp.tile([128, 16 * 128], F32, name="obs0")
        nc.sync.dma_start(out=t, in_=obs_v[0])
        obs_t.append(t)
        pol_sb = pap.tile([128, 512], F32)
        nc.sync.dma_start(out=pol_sb,
                          in_=pol.ap().rearrange("(p r) a -> p (r a)", p=128))
        act_sb = pap.tile([128, 512], F32)
        nc.sync.dma_start(out=act_sb,
                          in_=act.ap().rearrange("(p r) a -> p (r a)", p=128))
        for g in range(1, 3):
            t = obsp.tile([128, 16 * 128], F32, name=f"obs{g}")
            nc.sync.dma_start(out=t, in_=obs_v[g])
            obs_t.append(t)
        obs3_v = obs.ap().rearrange("(p s h nf) f -> s h p (nf f)",
                                    p=128, h=2, nf=8)
        obs3 = []
        for h in range(2):
            t = obsp.tile([128, 8 * 128], F32, name=f"obs3{h}")
            nc.sync.dma_start(out=t, in_=obs3_v[3][h])
            obs3.append(t)

        cst_sb = consts.tile([128, 10], F32)
        nc.scalar.dma_start(out=cst_sb, in_=cst.ap())
        cstb_sb = consts.tile([128, 452], BF16)
        nc.scalar.dma_start(out=cstb_sb, in_=cstb.ap())
        wvy_sb = cst_sb[:, 0:8]
        b1_sb = cst_sb[0:64, 8:9]
        biasq_sb = cst_sb[0:2, 9:10]
        idb_sb = cstb_sb[:, 0:128]
        w1q_sb = cstb_sb[:, 128:192]
        wq_sb = cstb_sb[0:64, 192:194]
        repw_sb = cstb_sb[0:2, 195:452]

        warm = consts.tile([1, 1], F32)
        nc.scalar.activation(out=warm, in_=cst_sb[0:1, 0:1], func=AFT.Sigmoid)

        wc8 = smp.tile([128, 8], F32)
        xw_v = xw.ap().rearrange("(p s d) j -> s p (d j)", p=128, d=16)
        xwbig = [smp.tile([128, 16, 2 * N], F32, name=f"xwbig_{g}")
                 for g in range(G)]

        def tree_full(g):
            src = obs_t[g]
            s1 = trp.tile([128, 1024], BF16, name="s1")
            nc.vector.tensor_add(s1, src[:, 0:1024], src[:, 1024:2048])
            s2 = trp.tile([128, 512], BF16, name="s2")
            nc.vector.tensor_add(s2, s1[:, 0:512], s1[:, 512:1024])
            s3 = trp.tile([128, 256], BF16, name="s3")
            nc.vector.tensor_add(s3, s2[:, 0:256], s2[:, 256:512])
            meanS = trp.tile([128, 128], BF16, name="meanS")
            last = nc.vector.tensor_add(meanS, s3[:, 0:128], s3[:, 128:256])
            return meanS, last

        def tree_half(h):
            src = obs3[h]
            h1 = trp.tile([128, 512], BF16, name="h1")
            nc.vector.tensor_add(h1, src[:, 0:512], src[:, 512:1024])
            h2 = trp.tile([128, 256], BF16, name="h2")
            nc.vector.tensor_add(h2, h1[:, 0:256], h1[:, 256:512])
            h3 = trp.tile([128, 128], BF16, name=f"h3_{h}")
            last = nc.vector.tensor_add(h3, h2[:, 0:128], h2[:, 128:256])
            return h3, last

        def chain(g, meanS):
            pmt = pmtp.tile([128, 128], BF16, name="pmt")
            nc.tensor.transpose(pmt, meanS[:], idb_sb)
            meanT = trp.tile([128, GE], BF16, name="meanT")
            nc.scalar.activation(out=meanT, in_=pmt, func=AFT.Copy)
            ph = php.tile([64, GE], F32, name="ph")
            nc.tensor.matmul(ph, lhsT=w1q_sb, rhs=meanT[:], start=True, stop=True)
            h_sb = trp.tile([64, GE], BF16, name="h_sb")
            nc.scalar.activation(out=h_sb, in_=ph, func=AFT.Relu, bias=b1_sb)
            pac = pacp.tile([2, GE], F32, name="pac")
            nc.tensor.matmul(pac, lhsT=wq_sb, rhs=h_sb, start=True, stop=True)
            wcb = trp.tile([2, GE], BF16, name="wcb")
            nc.scalar.activation(out=wcb, in_=pac, func=AFT.Identity, bias=biasq_sb)
            return wcb

        def lr_op(g, wcb):
            lr = trp.tile([1, GE], BF16, name="lr")
            i = nc.vector.scalar_tensor_tensor(out=lr, in0=wcb[0:1, :], scalar=0.01,
                                               in1=wcb[0:1, :], op0=ALU.mult,
                                               op1=ALU.max)
            return lr, i

        def post(g, wcb, lr):
            nc.scalar.activation(out=wcb[0:1, :], in_=lr, func=AFT.Sigmoid)
            pw = pwp.tile([128, 257], F32, name="pw")
            nc.tensor.matmul(pw, lhsT=wcb[:], rhs=repw_sb, start=True, stop=True)
            if g < 3:
                wc8_dst = bass.AP(tensor=wc8.tensor, offset=wc8.offset + g,
                                  ap=[wc8.ap[0], [4, 2]])
                pw2 = bass.AP(tensor=pw.tensor, offset=pw.offset,
                              ap=[pw.ap[0], [256, 2]])
                nc.scalar.activation(out=wc8_dst, in_=pw2, func=AFT.Copy)
            nc.scalar.activation(
                out=xwbig[g][:, :, N:2 * N],
                in_=pw[:, 0:256].rearrange("p (d j) -> p d j", j=16),
                func=AFT.Copy)
            return pw

        def prefix(g, eng, wcol=None, ccol=None):
            wcol = wc8[:, g:g + 1] if wcol is None else wcol
            ccol = wc8[:, 4 + g:5 + g] if ccol is None else ccol
            if eng is nc.vector:
                s1 = smp.tile([128, 1], F32, name=f"s1_{g}")
                eng.scalar_tensor_tensor(out=s1, in0=QS4[:, g:g + 1],
                                         scalar=wcol, in1=PS4[:, g:g + 1],
                                         op0=ALU.mult, op1=ALU.add)
                base = smp.tile([128, 1], F32, name=f"base_{g}")
                eng.scalar_tensor_tensor(out=base, in0=s1, scalar=1.0 / N,
                                         in1=ccol, op0=ALU.mult, op1=ALU.add)
            else:
                s1 = smp.tile([128, 1], F32, name=f"s1_{g}")
                eng.tensor_tensor(out=s1, in0=QS4[:, g:g + 1],
                                  in1=wcol, op=ALU.mult)
                eng.tensor_add(s1, s1, PS4[:, g:g + 1])
                base = smp.tile([128, 1], F32, name=f"base_{g}")
                eng.tensor_scalar_mul(base, s1, 1.0 / N)
                eng.tensor_add(base, base, ccol)
            negw = smp.tile([128, 1], F32, name=f"negw_{g}")
            eng.tensor_scalar_mul(negw, wcol, -1.0 / N)
            return base, negw

        def xcombine(g, base, negw):
            i = nc.vector.scalar_tensor_tensor(
                out=xwbig[g][:, :, 0:N],
                in0=Q64[:, 16 * g:16 * g + 16].unsqueeze(1)
                    .broadcast_to([128, 16, 16]),
                scalar=negw[:, 0:1],
                in1=base.unsqueeze(1).broadcast_to([128, 16, 16]),
                op0=ALU.mult, op1=ALU.add)
            nc.sync.dma_start(out=xw_v[g],
                              in_=xwbig[g].rearrange("p d j -> p (d j)"))
            return i

        m0, t0l = tree_full(0)
        wc0 = chain(0, m0)
        m1, t1l = tree_full(1)
        lr0, lr0i = lr_op(0, wc0)
        _dep(lr0i, t1l)
        post(0, wc0, lr0)
        wc1 = chain(1, m1)

        pol4 = pol_sb.rearrange("p (s r a) -> p s r a", s=G, a=8)
        act4 = act_sb.rearrange("p (s r a) -> p s r a", s=G, a=8)
        wvyb = wvy_sb.unsqueeze(1).unsqueeze(1).broadcast_to([128, G, 16, 8])
        tmP = gsp.tile([128, G, N * A], F32)
        nc.gpsimd.tensor_tensor(out=tmP.rearrange("p g (r a) -> p g r a", a=8),
                                in0=pol4, in1=wvyb, op=ALU.mult)
        tmA = gsp.tile([128, G, N * A], F32)
        nc.gpsimd.tensor_tensor(out=tmA.rearrange("p g (r a) -> p g r a", a=8),
                                in0=act4, in1=wvyb, op=ALU.mult)

        def gtree8(tm, nm):
            v = tm.rearrange("p g (r a) -> p (g r) a", a=8)
            t1 = gsp.tile([128, 64, 4], F32, name=f"t1{nm}")
            nc.gpsimd.tensor_add(t1, v[:, :, 0:4], v[:, :, 4:8])
            t2 = gsp.tile([128, 64, 2], F32, name=f"t2{nm}")
            nc.gpsimd.tensor_add(t2, t1[:, :, 0:2], t1[:, :, 2:4])
            t3 = gsp.tile([128, 64], F32, name=f"t3{nm}")
            nc.gpsimd.tensor_add(t3, t2[:, :, 0:1].rearrange("p i o -> p (i o)"),
                                 t2[:, :, 1:2].rearrange("p i o -> p (i o)"))
            return t3

        P64 = gtree8(tmP, "P")
        A64 = gtree8(tmA, "A")
        Q64 = gsp.tile([128, 64], F32)
        nc.gpsimd.tensor_sub(Q64, A64, P64)

        def gtree16(t64, nm):
            v = t64.rearrange("p (i n) -> p i n", n=16)
            u1 = gsp.tile([128, 4, 8], F32, name=f"u1{nm}")
            nc.gpsimd.tensor_add(u1, v[:, :, 0:8], v[:, :, 8:16])
            u2 = gsp.tile([128, 4, 4], F32, name=f"u2{nm}")
            nc.gpsimd.tensor_add(u2, u1[:, :, 0:4], u1[:, :, 4:8])
            u3 = gsp.tile([128, 4, 2], F32, name=f"u3{nm}")
            nc.gpsimd.tensor_add(u3, u2[:, :, 0:2], u2[:, :, 2:4])
            u4 = gsp.tile([128, 4], F32, name=f"u4{nm}")
            nc.gpsimd.tensor_add(u4, u3[:, :, 0:1].rearrange("p i o -> p (i o)"),
                                 u3[:, :, 1:2].rearrange("p i o -> p (i o)"))
            return u4

        PS4 = gtree16(P64, "P")
        QS4 = gtree16(Q64, "Q")

        m2, t2l = tree_full(2)
        lr1, lr1i = lr_op(1, wc1)
        _dep(lr1i, t2l)
        post(1, wc1, lr1)
        wc2 = chain(2, m2)
        a33, a33l = tree_half(0)
        lr2, lr2i = lr_op(2, wc2)
        _dep(lr2i, a33l)
        post(2, wc2, lr2)
        b33, b33l = tree_half(1)
        m3 = trp.tile([128, 128], BF16, name="meanS3")
        t3l = nc.vector.tensor_add(m3, a33, b33)
        bn0 = prefix(0, nc.gpsimd)
        bn1 = prefix(1, nc.gpsimd)
        bn2 = prefix(2, nc.gpsimd)
        # group-3 tail refactor: xv = w*(QS-Q)/16 + (c + PS/16)
        qd16 = smp.tile([128, 1], F32)
        nc.gpsimd.tensor_scalar_mul(qd16, QS4[:, 3:4], 1.0 / N)
        psd16 = smp.tile([128, 1], F32)
        nc.gpsimd.tensor_scalar_mul(psd16, PS4[:, 3:4], 1.0 / N)
        U16 = smp.tile([128, 16], F32)
        u16i = nc.vector.scalar_tensor_tensor(
            out=U16, in0=Q64[:, 48:64], scalar=-1.0 / N,
            in1=qd16.broadcast_to([128, 16]), op0=ALU.mult, op1=ALU.add)
        _dep(u16i, t3l)
        x0i = xcombine(0, *bn0)
        _dep(x0i, t3l)
        x1i = xcombine(1, *bn1)
        _dep(x1i, t3l)
        x2i = xcombine(2, *bn2)
        _dep(x2i, t3l)
        wc3 = chain(3, m3)
        lr3, lr3i = lr_op(3, wc3)
        _dep(lr3i, t3l)
        pw3 = post(3, wc3, lr3)
        cps = smp.tile([128, 1], F32)
        nc.vector.tensor_add(cps, pw3[:, 256:257], psd16)
        nc.vector.scalar_tensor_tensor(
            out=xwbig[3][:, :, 0:N],
            in0=U16.unsqueeze(1).broadcast_to([128, 16, 16]),
            scalar=pw3[:, 0:1],
            in1=cps.unsqueeze(1).broadcast_to([128, 16, 16]),
            op0=ALU.mult, op1=ALU.add)
        # ship group 3 as two d-halves on both rings concurrently
        xwf3 = xwbig[3].rearrange("p d j -> p (d j)")
        nc.sync.dma_start(out=xw_v[3][:, 0:256], in_=xwf3[:, 0:256])
        nc.scalar.dma_start(out=xw_v[3][:, 256:512], in_=xwf3[:, 256:512])

    nc.compile()
    return nc


_NC_CACHE = {}


def _get_nc():
    if "nc" not in _NC_CACHE:
        _NC_CACHE["nc"] = _build()
    return _NC_CACHE["nc"]


def _make_in_maps(inputs):
    obs = np.ascontiguousarray(np.asarray(inputs["obs"], np.float32))
    pol = np.ascontiguousarray(np.asarray(inputs["policies"], np.float32))
    act = np.ascontiguousarray(np.asarray(inputs["actions"], np.float32))
    W1 = np.asarray(inputs["W1"], np.float32)
    b1 = np.asarray(inputs["b1"], np.float32)
    W2 = np.asarray(inputs["W2"], np.float32)
    b2 = np.asarray(inputs["b2"], np.float32)
    Wfc = np.asarray(inputs["Wfc"], np.float32)
    Wattn = np.asarray(inputs["Wattn"], np.float32)
    Wv = np.asarray(inputs["Wv"], np.float32)
    bv = np.asarray(inputs["bv"], np.float32)

    wa = (Wfc @ (Wattn[:DZ] + Wattn[DZ:]))[:, 0]
    wvy = Wv[DP:, 0]
    wv64 = Wv[:DP, 0]

    cst = np.zeros((128, 10), np.float32)
    cst[:, 0:8] = wvy[None, :]
    cst[0:64, 8] = b1
    cst[0, 9] = float(b2 @ wa)
    cst[1, 9] = float(b2 @ wv64 + bv[0])
    cstb = np.zeros((128, 452), np.float32)
    cstb[:, 0:128] = np.eye(128, dtype=np.float32)
    cstb[:, 128:192] = W1 / 16.0
    cstb[0:64, 192] = W2 @ wa
    cstb[0:64, 193] = W2 @ wv64
    cstb[0, 195:451] = 1.0
    cstb[1, 451] = 1.0
    cstb = cstb.astype(ml_dtypes.bfloat16)

    in_maps = []
    for c in range(NCORES):
        in_maps.append({
            "obs": obs[c * RC:(c + 1) * RC],
            "pol": pol[c * RC:(c + 1) * RC],
            "act": act[c * RC:(c + 1) * RC],
            "cst": cst,
            "cstb": cstb,
        })
    return in_maps


TRACE = False
TRACE_KWARGS = {}
LAST_RESULT = None


def kernel(**inputs):
    global LAST_RESULT
    nc = _get_nc()
    in_maps = _make_in_maps(inputs)
    res = run_bass_kernel_spmd(nc, in_maps, core_ids=list(range(NCORES)),
                               trace=TRACE, **TRACE_KWARGS)
    LAST_RESULT = res
    xwf = np.concatenate([r["xw"] for r in res.results], axis=0)
    x = np.ascontiguousarray(xwf[:, 0:N]).reshape(B * N, N, 1)
    w = np.ascontiguousarray(xwf[:, N:2 * N]).reshape(B * N, N, 1)
    return x, w


# revision 34
# speedup vs baseline: 1.1214x; 1.1214x over previous
"""v2.4 reconstruction: best-measured config (36793 ns).

q1: obs0, pol, act, obs1, obs2, obs3A, obs3B, xw0-3.  q10: cst, cstb.
P/A entirely on gpsimd; trees/lr/combines on DVE; chains on PE/ACT;
prefix0-2 gpsimd, prefix3 DVE from the pw PSUM columns.
"""

import numpy as np
import ml_dtypes
from contextlib import ExitStack

import concourse.bass as bass
import concourse.bacc as bacc
import concourse.tile as tile
from concourse import mybir
from concourse.bass_utils import run_bass_kernel_spmd

B, N, A = 4096, 16, 8
D_IN, H1, DP, DZ = 128, 64, 64, 64
NCORES = 8
BC = B // NCORES
RC = BC * N
G = 4
GE = BC // G

F32 = mybir.dt.float32
BF16 = mybir.dt.bfloat16
ALU = mybir.AluOpType
AFT = mybir.ActivationFunctionType


def _dep(later, earlier):
    if later is not None and earlier is not None:
        tile.add_dep_helper(later.ins, earlier.ins, sync=False,
                            reason="engine stream order")


def _build():
    nc = bacc.Bacc("TRN2", target_bir_lowering=False, debug=False)

    obs = nc.dram_tensor("obs", [RC, D_IN], F32, kind="ExternalInput")
    pol = nc.dram_tensor("pol", [RC, A], F32, kind="ExternalInput")
    act = nc.dram_tensor("act", [RC, A], F32, kind="ExternalInput")
    cst = nc.dram_tensor("cst", [128, 10], F32, kind="ExternalInput")
    cstb = nc.dram_tensor("cstb", [128, 452], BF16, kind="ExternalInput")
    xw = nc.dram_tensor("xw", [RC, 2 * N], F32, kind="ExternalOutput")

    with ExitStack() as ctx:
        tc = ctx.enter_context(tile.TileContext(nc))
        consts = ctx.enter_context(tc.tile_pool(name="consts", bufs=1))
        obsp = ctx.enter_context(tc.tile_pool(name="obsp", bufs=1))
        trp = ctx.enter_context(tc.tile_pool(name="trp", bufs=2))
        pap = ctx.enter_context(tc.tile_pool(name="pap", bufs=1))
        gsp = ctx.enter_context(tc.tile_pool(name="gsp", bufs=1))
        smp = ctx.enter_context(tc.tile_pool(name="smp", bufs=1))
        pmtp = ctx.enter_context(tc.tile_pool(name="pmtp", bufs=2, space="PSUM"))
        php = ctx.enter_context(tc.tile_pool(name="php", bufs=2, space="PSUM"))
        pacp = ctx.enter_context(tc.tile_pool(name="pacp", bufs=2, space="PSUM"))
        pwp = ctx.enter_context(tc.tile_pool(name="pwp", bufs=2, space="PSUM"))

        obs_v = obs.ap().rearrange("(p s n) f -> s p (n f)", p=128, n=16)
        obs_t = []
        t = obsp.tile([128, 16 * 128], F32, name="obs0")
        nc.sync.dma_start(out=t, in_=obs_v[0])
        obs_t.append(t)
        pol_sb = pap.tile([128, 512], F32)
        nc.sync.dma_start(out=pol_sb,
                          in_=pol.ap().rearrange("(p r) a -> p (r a)", p=128))
        act_sb = pap.tile([128, 512], F32)
        nc.sync.dma_start(out=act_sb,
                          in_=act.ap().rearrange("(p r) a -> p (r a)", p=128))
        for g in range(1, 3):
            t = obsp.tile([128, 16 * 128], F32, name=f"obs{g}")
            nc.sync.dma_start(out=t, in_=obs_v[g])
            obs_t.append(t)
        obs3_v = obs.ap().rearrange("(p s h nf) f -> s h p (nf f)",
                                    p=128, h=2, nf=8)
        obs3 = []
        for h in range(2):
            t = obsp.tile([128, 8 * 128], F32, name=f"obs3{h}")
            nc.sync.dma_start(out=t, in_=obs3_v[3][h])
            obs3.append(t)

        cst_sb = consts.tile([128, 10], F32)
        nc.scalar.dma_start(out=cst_sb, in_=cst.ap())
        cstb_sb = consts.tile([128, 452], BF16)
        nc.scalar.dma_start(out=cstb_sb, in_=cstb.ap())
        wvy_sb = cst_sb[:, 0:8]
        b1_sb = cst_sb[0:64, 8:9]
        biasq_sb = cst_sb[0:2, 9:10]
        idb_sb = cstb_sb[:, 0:128]
        w1q_sb = cstb_sb[:, 128:192]
        wq_sb = cstb_sb[0:64, 192:194]
        repw_sb = cstb_sb[0:2, 195:452]

        warm = consts.tile([1, 1], F32)
        nc.scalar.activation(out=warm, in_=cst_sb[0:1, 0:1], func=AFT.Sigmoid)

        wc8 = smp.tile([128, 8], F32)
        xw_v = xw.ap().rearrange("(p s d) j -> s p (d j)", p=128, d=16)
        xwbig = [smp.tile([128, 16, 2 * N], F32, name=f"xwbig_{g}")
                 for g in range(G)]

        def tree_full(g):
            src = obs_t[g]
            s1 = trp.tile([128, 1024], BF16, name="s1")
            nc.vector.tensor_add(s1, src[:, 0:1024], src[:, 1024:2048])
            s2 = trp.tile([128, 512], BF16, name="s2")
            nc.vector.tensor_add(s2, s1[:, 0:512], s1[:, 512:1024])
            s3 = trp.tile([128, 256], BF16, name="s3")
            nc.vector.tensor_add(s3, s2[:, 0:256], s2[:, 256:512])
            meanS = trp.tile([128, 128], BF16, name="meanS")
            last = nc.vector.tensor_add(meanS, s3[:, 0:128], s3[:, 128:256])
            return meanS, last

        def tree_half(h):
            src = obs3[h]
            h1 = trp.tile([128, 512], BF16, name="h1")
            nc.vector.tensor_add(h1, src[:, 0:512], src[:, 512:1024])
            h2 = trp.tile([128, 256], BF16, name="h2")
            nc.vector.tensor_add(h2, h1[:, 0:256], h1[:, 256:512])
            h3 = trp.tile([128, 128], BF16, name=f"h3_{h}")
            last = nc.vector.tensor_add(h3, h2[:, 0:128], h2[:, 128:256])
            return h3, last

        def chain(g, meanS):
            pmt = pmtp.tile([128, 128], BF16, name="pmt")
            nc.tensor.transpose(pmt, meanS[:], idb_sb)
            meanT = trp.tile([128, GE], BF16, name="meanT")
            nc.scalar.activation(out=meanT, in_=pmt, func=AFT.Copy)
            ph = php.tile([64, GE], F32, name="ph")
            nc.tensor.matmul(ph, lhsT=w1q_sb, rhs=meanT[:], start=True, stop=True)
            h_sb = trp.tile([64, GE], BF16, name="h_sb")
            nc.scalar.activation(out=h_sb, in_=ph, func=AFT.Relu, bias=b1_sb)
            pac = pacp.tile([2, GE], F32, name="pac")
            nc.tensor.matmul(pac, lhsT=wq_sb, rhs=h_sb, start=True, stop=True)
            wcb = trp.tile([2, GE], BF16, name="wcb")
            nc.scalar.activation(out=wcb, in_=pac, func=AFT.Identity, bias=biasq_sb)
            return wcb

        def lr_op(g, wcb):
            lr = trp.tile([1, GE], BF16, name="lr")
            i = nc.vector.scalar_tensor_tensor(out=lr, in0=wcb[0:1, :], scalar=0.01,
                                               in1=wcb[0:1, :], op0=ALU.mult,
                                               op1=ALU.max)
            return lr, i

        def post(g, wcb, lr):
            nc.scalar.activation(out=wcb[0:1, :], in_=lr, func=AFT.Sigmoid)
            pw = pwp.tile([128, 257], F32, name="pw")
            nc.tensor.matmul(pw, lhsT=wcb[:], rhs=repw_sb, start=True, stop=True)
            if g < 3:
                wc8_dst = bass.AP(tensor=wc8.tensor, offset=wc8.offset + g,
                                  ap=[wc8.ap[0], [4, 2]])
                pw2 = bass.AP(tensor=pw.tensor, offset=pw.offset,
                              ap=[pw.ap[0], [256, 2]])
                nc.scalar.activation(out=wc8_dst, in_=pw2, func=AFT.Copy)
            nc.scalar.activation(
                out=xwbig[g][:, :, N:2 * N],
                in_=pw[:, 0:256].rearrange("p (d j) -> p d j", j=16),
                func=AFT.Copy)
            return pw

        def prefix(g, eng, wcol=None, ccol=None):
            wcol = wc8[:, g:g + 1] if wcol is None else wcol
            ccol = wc8[:, 4 + g:5 + g] if ccol is None else ccol
            if eng is nc.vector:
                s1 = smp.tile([128, 1], F32, name=f"s1_{g}")
                eng.scalar_tensor_tensor(out=s1, in0=QS4[:, g:g + 1],
                                         scalar=wcol, in1=PS4[:, g:g + 1],
                                         op0=ALU.mult, op1=ALU.add)
                base = smp.tile([128, 1], F32, name=f"base_{g}")
                eng.scalar_tensor_tensor(out=base, in0=s1, scalar=1.0 / N,
                                         in1=ccol, op0=ALU.mult, op1=ALU.add)
            else:
                s1 = smp.tile([128, 1], F32, name=f"s1_{g}")
                eng.tensor_tensor(out=s1, in0=QS4[:, g:g + 1],
                                  in1=wcol, op=ALU.mult)
                eng.tensor_add(s1, s1, PS4[:, g:g + 1])
                base = smp.tile([128, 1], F32, name=f"base_{g}")
                eng.tensor_scalar_mul(base, s1, 1.0 / N)
                eng.tensor_add(base, base, ccol)
            negw = smp.tile([128, 1], F32, name=f"negw_{g}")
            eng.tensor_scalar_mul(negw, wcol, -1.0 / N)
            return base, negw

        def xcombine(g, base, negw):
            i = nc.vector.scalar_tensor_tensor(
                out=xwbig[g][:, :, 0:N],
                in0=Q64[:, 16 * g:16 * g + 16].unsqueeze(1)
                    .broadcast_to([128, 16, 16]),
                scalar=negw[:, 0:1],
                in1=base.unsqueeze(1).broadcast_to([128, 16, 16]),
                op0=ALU.mult, op1=ALU.add)
            nc.sync.dma_start(out=xw_v[g],
                              in_=xwbig[g].rearrange("p d j -> p (d j)"))
            return i

        m0, t0l = tree_full(0)
        wc0 = chain(0, m0)
        m1, t1l = tree_full(1)
        lr0, lr0i = lr_op(0, wc0)
        _dep(lr0i, t1l)
        post(0, wc0, lr0)
        wc1 = chain(1, m1)

        pol4 = pol_sb.rearrange("p (s r a) -> p s r a", s=G, a=8)
        act4 = act_sb.rearrange("p (s r a) -> p s r a", s=G, a=8)
        wvyb = wvy_sb.unsqueeze(1).unsqueeze(1).broadcast_to([128, G, 16, 8])
        tmP = gsp.tile([128, G, N * A], F32)
        nc.gpsimd.tensor_tensor(out=tmP.rearrange("p g (r a) -> p g r a", a=8),
                                in0=pol4, in1=wvyb, op=ALU.mult)
        tmA = gsp.tile([128, G, N * A], F32)
        nc.gpsimd.tensor_tensor(out=tmA.rearrange("p g (r a) -> p g r a", a=8),
                                in0=act4, in1=wvyb, op=ALU.mult)

        def gtree8(tm, nm):
            v = tm.rearrange("p g (r a) -> p (g r) a", a=8)
            t1 = gsp.tile([128, 64, 4], F32, name=f"t1{nm}")
            nc.gpsimd.tensor_add(t1, v[:, :, 0:4], v[:, :, 4:8])
            t2 = gsp.tile([128, 64, 2], F32, name=f"t2{nm}")
            nc.gpsimd.tensor_add(t2, t1[:, :, 0:2], t1[:, :, 2:4])
            t3 = gsp.tile([128, 64], F32, name=f"t3{nm}")
            nc.gpsimd.tensor_add(t3, t2[:, :, 0:1].rearrange("p i o -> p (i o)"),
                                 t2[:, :, 1:2].rearrange("p i o -> p (i o)"))
            return t3

        P64 = gtree8(tmP, "P")
        A64 = gtree8(tmA, "A")
        Q64 = gsp.tile([128, 64], F32)
        nc.gpsimd.tensor_sub(Q64, A64, P64)

        def gtree16(t64, nm):
            v = t64.rearrange("p (i n) -> p i n", n=16)
            u1 = gsp.tile([128, 4, 8], F32, name=f"u1{nm}")
            nc.gpsimd.tensor_add(u1, v[:, :, 0:8], v[:, :, 8:16])
            u2 = gsp.tile([128, 4, 4], F32, name=f"u2{nm}")
            nc.gpsimd.tensor_add(u2, u1[:, :, 0:4], u1[:, :, 4:8])
            u3 = gsp.tile([128, 4, 2], F32, name=f"u3{nm}")
            nc.gpsimd.tensor_add(u3, u2[:, :, 0:2], u2[:, :, 2:4])
            u4 = gsp.tile([128, 4], F32, name=f"u4{nm}")
            nc.gpsimd.tensor_add(u4, u3[:, :, 0:1].rearrange("p i o -> p (i o)"),
                                 u3[:, :, 1:2].rearrange("p i o -> p (i o)"))
            return u4

        PS4 = gtree16(P64, "P")
        QS4 = gtree16(Q64, "Q")

        m2, t2l = tree_full(2)
        lr1, lr1i = lr_op(1, wc1)
        _dep(lr1i, t2l)
        post(1, wc1, lr1)
        wc2 = chain(2, m2)
        a33, a33l = tree_half(0)
        lr2, lr2i = lr_op(2, wc2)
        _dep(lr2i, a33l)
        post(2, wc2, lr2)
        b33, b33l = tree_half(1)
        m3 = trp.tile([128, 128], BF16, name="meanS3")
        t3l = nc.vector.tensor_add(m3, a33, b33)
        # all groups: xv = w*(QS-Q)/16 + (c + PS/16); the w/c-independent
        # parts precompute as soon as Q64/QS4/PS4 exist
        qd16 = smp.tile([128, 4], F32)
        nc.gpsimd.tensor_scalar_mul(qd16, QS4, 1.0 / N)
        psd16 = smp.tile([128, 4], F32)
        nc.gpsimd.tensor_scalar_mul(psd16, PS4, 1.0 / N)
        U16 = smp.tile([128, 4, 16], F32)
        u16i = nc.vector.scalar_tensor_tensor(
            out=U16, in0=Q64.rearrange("p (i n) -> p i n", n=16),
            scalar=-1.0 / N,
            in1=qd16.unsqueeze(2).broadcast_to([128, 4, 16]),
            op0=ALU.mult, op1=ALU.add)
        _dep(u16i, t3l)

        def xcombine2(g, wcol, cps_g):
            i = nc.vector.scalar_tensor_tensor(
                out=xwbig[g][:, :, 0:N],
                in0=U16[:, g].unsqueeze(1).broadcast_to([128, 16, 16]),
                scalar=wcol,
                in1=cps_g.unsqueeze(1).broadcast_to([128, 16, 16]),
                op0=ALU.mult, op1=ALU.add)
            nc.sync.dma_start(out=xw_v[g],
                              in_=xwbig[g].rearrange("p d j -> p (d j)"))
            return i

        for g in range(3):
            cps_g = smp.tile([128, 1], F32, name=f"cps_{g}")
            nc.gpsimd.tensor_add(cps_g, wc8[:, 4 + g:5 + g],
                                 psd16[:, g:g + 1])
            xgi = xcombine2(g, wc8[:, g:g + 1], cps_g)
            _dep(xgi, t3l)
        wc3 = chain(3, m3)
        lr3, lr3i = lr_op(3, wc3)
        _dep(lr3i, t3l)
        pw3 = post(3, wc3, lr3)
        cps = smp.tile([128, 1], F32)
        nc.vector.tensor_add(cps, pw3[:, 256:257], psd16[:, 3:4])
        nc.vector.scalar_tensor_tensor(
            out=xwbig[3][:, :, 0:N],
            in0=U16[:, 3].unsqueeze(1).broadcast_to([128, 16, 16]),
            scalar=pw3[:, 0:1],
            in1=cps.unsqueeze(1).broadcast_to([128, 16, 16]),
            op0=ALU.mult, op1=ALU.add)
        # ship group 3 as two d-halves on both rings concurrently
        xwf3 = xwbig[3].rearrange("p d j -> p (d j)")
        nc.sync.dma_start(out=xw_v[3][:, 0:256], in_=xwf3[:, 0:256])
        nc.scalar.dma_start(out=xw_v[3][:, 256:512], in_=xwf3[:, 256:512])

    nc.compile()
    return nc


_NC_CACHE = {}


def _get_nc():
    if "nc" not in _NC_CACHE:
        _NC_CACHE["nc"] = _build()
    return _NC_CACHE["nc"]


def _make_in_maps(inputs):
    obs = np.ascontiguousarray(np.asarray(inputs["obs"], np.float32))
    pol = np.ascontiguousarray(np.asarray(inputs["policies"], np.float32))
    act = np.ascontiguousarray(np.asarray(inputs["actions"], np.float32))
    W1 = np.asarray(inputs["W1"], np.float32)
    b1 = np.asarray(inputs["b1"], np.float32)
    W2 = np.asarray(inputs["W2"], np.float32)
    b2 = np.asarray(inputs["b2"], np.float32)
    Wfc = np.asarray(inputs["Wfc"], np.float32)
    Wattn = np.asarray(inputs["Wattn"], np.float32)
    Wv = np.asarray(inputs["Wv"], np.float32)
    bv = np.asarray(inputs["bv"], np.float32)

    wa = (Wfc @ (Wattn[:DZ] + Wattn[DZ:]))[:, 0]
    wvy = Wv[DP:, 0]
    wv64 = Wv[:DP, 0]

    cst = np.zeros((128, 10), np.float32)
    cst[:, 0:8] = wvy[None, :]
    cst[0:64, 8] = b1
    cst[0, 9] = float(b2 @ wa)
    cst[1, 9] = float(b2 @ wv64 + bv[0])
    cstb = np.zeros((128, 452), np.float32)
    cstb[:, 0:128] = np.eye(128, dtype=np.float32)
    cstb[:, 128:192] = W1 / 16.0
    cstb[0:64, 192] = W2 @ wa
    cstb[0:64, 193] = W2 @ wv64
    cstb[0, 195:451] = 1.0
    cstb[1, 451] = 1.0
    cstb = cstb.astype(ml_dtypes.bfloat16)

    in_maps = []
    for c in range(NCORES):
        in_maps.append({
            "obs": obs[c * RC:(c + 1) * RC],
            "pol": pol[c * RC:(c + 1) * RC],
            "act": act[c * RC:(c + 1) * RC],
            "cst": cst,
            "cstb": cstb,
        })
    return in_maps


TRACE = False
TRACE_KWARGS = {}
LAST_RESULT = None


def kernel(**inputs):
    global LAST_RESULT
    nc = _get_nc()
    in_maps = _make_in_maps(inputs)
    res = run_bass_kernel_spmd(nc, in_maps, core_ids=list(range(NCORES)),
                               trace=TRACE, **TRACE_KWARGS)
    LAST_RESULT = res
    xwf = np.concatenate([r["xw"] for r in res.results], axis=0)
    x = np.ascontiguousarray(xwf[:, 0:N]).reshape(B * N, N, 1)
    w = np.ascontiguousarray(xwf[:, N:2 * N]).reshape(B * N, N, 1)
    return x, w


# revision 35
# speedup vs baseline: 1.1652x; 1.0390x over previous
"""v2.4 reconstruction: best-measured config (36793 ns).

q1: obs0, pol, act, obs1, obs2, obs3A, obs3B, xw0-3.  q10: cst, cstb.
P/A entirely on gpsimd; trees/lr/combines on DVE; chains on PE/ACT;
prefix0-2 gpsimd, prefix3 DVE from the pw PSUM columns.
"""

import numpy as np
import ml_dtypes
from contextlib import ExitStack

import concourse.bass as bass
import concourse.bacc as bacc
import concourse.tile as tile
from concourse import mybir
from concourse.bass_utils import run_bass_kernel_spmd

B, N, A = 4096, 16, 8
D_IN, H1, DP, DZ = 128, 64, 64, 64
NCORES = 8
BC = B // NCORES
RC = BC * N
G = 4
GE = BC // G

F32 = mybir.dt.float32
BF16 = mybir.dt.bfloat16
ALU = mybir.AluOpType
AFT = mybir.ActivationFunctionType


def _dep(later, earlier):
    if later is not None and earlier is not None:
        tile.add_dep_helper(later.ins, earlier.ins, sync=False,
                            reason="engine stream order")


def _build():
    nc = bacc.Bacc("TRN2", target_bir_lowering=False, debug=False)

    obs = nc.dram_tensor("obs", [RC, D_IN], F32, kind="ExternalInput")
    pol = nc.dram_tensor("pol", [RC, A], F32, kind="ExternalInput")
    act = nc.dram_tensor("act", [RC, A], F32, kind="ExternalInput")
    cst = nc.dram_tensor("cst", [128, 10], F32, kind="ExternalInput")
    cstb = nc.dram_tensor("cstb", [128, 452], BF16, kind="ExternalInput")
    xw = nc.dram_tensor("xw", [RC, 2 * N], F32, kind="ExternalOutput")

    with ExitStack() as ctx:
        tc = ctx.enter_context(tile.TileContext(nc))
        consts = ctx.enter_context(tc.tile_pool(name="consts", bufs=1))
        obsp = ctx.enter_context(tc.tile_pool(name="obsp", bufs=1))
        trp = ctx.enter_context(tc.tile_pool(name="trp", bufs=2))
        pap = ctx.enter_context(tc.tile_pool(name="pap", bufs=1))
        gsp = ctx.enter_context(tc.tile_pool(name="gsp", bufs=1))
        smp = ctx.enter_context(tc.tile_pool(name="smp", bufs=1))
        pmtp = ctx.enter_context(tc.tile_pool(name="pmtp", bufs=2, space="PSUM"))
        php = ctx.enter_context(tc.tile_pool(name="php", bufs=2, space="PSUM"))
        pacp = ctx.enter_context(tc.tile_pool(name="pacp", bufs=2, space="PSUM"))
        pwp = ctx.enter_context(tc.tile_pool(name="pwp", bufs=2, space="PSUM"))

        obs_v = obs.ap().rearrange("(p s n) f -> s p (n f)", p=128, n=16)
        obs_t = []
        t = obsp.tile([128, 16 * 128], F32, name="obs0")
        nc.sync.dma_start(out=t, in_=obs_v[0])
        obs_t.append(t)
        pol_sb = pap.tile([128, 512], F32)
        nc.sync.dma_start(out=pol_sb,
                          in_=pol.ap().rearrange("(p r) a -> p (r a)", p=128))
        act_sb = pap.tile([128, 512], F32)
        nc.sync.dma_start(out=act_sb,
                          in_=act.ap().rearrange("(p r) a -> p (r a)", p=128))
        for g in range(1, 3):
            t = obsp.tile([128, 16 * 128], F32, name=f"obs{g}")
            nc.sync.dma_start(out=t, in_=obs_v[g])
            obs_t.append(t)
        t = obsp.tile([128, 16 * 128], F32, name="obs3")
        nc.sync.dma_start(out=t, in_=obs_v[3])
        obs_t.append(t)

        cst_sb = consts.tile([128, 10], F32)
        nc.scalar.dma_start(out=cst_sb, in_=cst.ap())
        cstb_sb = consts.tile([128, 452], BF16)
        nc.scalar.dma_start(out=cstb_sb, in_=cstb.ap())
        wvy_sb = cst_sb[:, 0:8]
        b1_sb = cst_sb[0:64, 8:9]
        biasq_sb = cst_sb[0:2, 9:10]
        idb_sb = cstb_sb[:, 0:128]
        w1q_sb = cstb_sb[:, 128:192]
        wq_sb = cstb_sb[0:64, 192:194]
        repw_sb = cstb_sb[0:2, 195:452]

        warm = consts.tile([1, 1], F32)
        nc.scalar.activation(out=warm, in_=cst_sb[0:1, 0:1], func=AFT.Sigmoid)

        wc8 = smp.tile([128, 8], F32)
        xw_v = xw.ap().rearrange("(p s d) j -> s p (d j)", p=128, d=16)
        xwbig = [smp.tile([128, 16, 2 * N], F32, name=f"xwbig_{g}")
                 for g in range(G)]

        def tree_full(g):
            src = obs_t[g]
            s1 = trp.tile([128, 1024], BF16, name="s1")
            nc.vector.tensor_add(s1, src[:, 0:1024], src[:, 1024:2048])
            s2 = trp.tile([128, 512], BF16, name="s2")
            nc.vector.tensor_add(s2, s1[:, 0:512], s1[:, 512:1024])
            s3 = trp.tile([128, 256], BF16, name="s3")
            nc.vector.tensor_add(s3, s2[:, 0:256], s2[:, 256:512])
            meanS = trp.tile([128, 128], BF16, name="meanS")
            last = nc.vector.tensor_add(meanS, s3[:, 0:128], s3[:, 128:256])
            return meanS, last

        def tree_half(h):
            src = obs3[h]
            h1 = trp.tile([128, 512], BF16, name="h1")
            nc.vector.tensor_add(h1, src[:, 0:512], src[:, 512:1024])
            h2 = trp.tile([128, 256], BF16, name="h2")
            nc.vector.tensor_add(h2, h1[:, 0:256], h1[:, 256:512])
            h3 = trp.tile([128, 128], BF16, name=f"h3_{h}")
            last = nc.vector.tensor_add(h3, h2[:, 0:128], h2[:, 128:256])
            return h3, last

        def chain(g, meanS):
            pmt = pmtp.tile([128, 128], BF16, name="pmt")
            nc.tensor.transpose(pmt, meanS[:], idb_sb)
            meanT = trp.tile([128, GE], BF16, name="meanT")
            nc.scalar.activation(out=meanT, in_=pmt, func=AFT.Copy)
            ph = php.tile([64, GE], F32, name="ph")
            nc.tensor.matmul(ph, lhsT=w1q_sb, rhs=meanT[:], start=True, stop=True)
            h_sb = trp.tile([64, GE], BF16, name="h_sb")
            nc.scalar.activation(out=h_sb, in_=ph, func=AFT.Relu, bias=b1_sb)
            pac = pacp.tile([2, GE], F32, name="pac")
            nc.tensor.matmul(pac, lhsT=wq_sb, rhs=h_sb, start=True, stop=True)
            wcb = trp.tile([2, GE], BF16, name="wcb")
            nc.scalar.activation(out=wcb, in_=pac, func=AFT.Identity, bias=biasq_sb)
            return wcb

        def lr_op(g, wcb):
            lr = trp.tile([1, GE], BF16, name="lr")
            i = nc.vector.scalar_tensor_tensor(out=lr, in0=wcb[0:1, :], scalar=0.01,
                                               in1=wcb[0:1, :], op0=ALU.mult,
                                               op1=ALU.max)
            return lr, i

        def post(g, wcb, lr):
            nc.scalar.activation(out=wcb[0:1, :], in_=lr, func=AFT.Sigmoid)
            pw = pwp.tile([128, 257], F32, name="pw")
            nc.tensor.matmul(pw, lhsT=wcb[:], rhs=repw_sb, start=True, stop=True)
            if g < 3:
                wc8_dst = bass.AP(tensor=wc8.tensor, offset=wc8.offset + g,
                                  ap=[wc8.ap[0], [4, 2]])
                pw2 = bass.AP(tensor=pw.tensor, offset=pw.offset,
                              ap=[pw.ap[0], [256, 2]])
                nc.scalar.activation(out=wc8_dst, in_=pw2, func=AFT.Copy)
            nc.scalar.activation(
                out=xwbig[g][:, :, N:2 * N],
                in_=pw[:, 0:256].rearrange("p (d j) -> p d j", j=16),
                func=AFT.Copy)
            return pw

        def prefix(g, eng, wcol=None, ccol=None):
            wcol = wc8[:, g:g + 1] if wcol is None else wcol
            ccol = wc8[:, 4 + g:5 + g] if ccol is None else ccol
            if eng is nc.vector:
                s1 = smp.tile([128, 1], F32, name=f"s1_{g}")
                eng.scalar_tensor_tensor(out=s1, in0=QS4[:, g:g + 1],
                                         scalar=wcol, in1=PS4[:, g:g + 1],
                                         op0=ALU.mult, op1=ALU.add)
                base = smp.tile([128, 1], F32, name=f"base_{g}")
                eng.scalar_tensor_tensor(out=base, in0=s1, scalar=1.0 / N,
                                         in1=ccol, op0=ALU.mult, op1=ALU.add)
            else:
                s1 = smp.tile([128, 1], F32, name=f"s1_{g}")
                eng.tensor_tensor(out=s1, in0=QS4[:, g:g + 1],
                                  in1=wcol, op=ALU.mult)
                eng.tensor_add(s1, s1, PS4[:, g:g + 1])
                base = smp.tile([128, 1], F32, name=f"base_{g}")
                eng.tensor_scalar_mul(base, s1, 1.0 / N)
                eng.tensor_add(base, base, ccol)
            negw = smp.tile([128, 1], F32, name=f"negw_{g}")
            eng.tensor_scalar_mul(negw, wcol, -1.0 / N)
            return base, negw

        def xcombine(g, base, negw):
            i = nc.vector.scalar_tensor_tensor(
                out=xwbig[g][:, :, 0:N],
                in0=Q64[:, 16 * g:16 * g + 16].unsqueeze(1)
                    .broadcast_to([128, 16, 16]),
                scalar=negw[:, 0:1],
                in1=base.unsqueeze(1).broadcast_to([128, 16, 16]),
                op0=ALU.mult, op1=ALU.add)
            nc.sync.dma_start(out=xw_v[g],
                              in_=xwbig[g].rearrange("p d j -> p (d j)"))
            return i

        m0, t0l = tree_full(0)
        wc0 = chain(0, m0)
        m1, t1l = tree_full(1)
        lr0, lr0i = lr_op(0, wc0)
        _dep(lr0i, t1l)
        post(0, wc0, lr0)
        wc1 = chain(1, m1)

        pol4 = pol_sb.rearrange("p (s r a) -> p s r a", s=G, a=8)
        act4 = act_sb.rearrange("p (s r a) -> p s r a", s=G, a=8)
        wvyb = wvy_sb.unsqueeze(1).unsqueeze(1).broadcast_to([128, G, 16, 8])
        tmP = gsp.tile([128, G, N * A], F32)
        nc.gpsimd.tensor_tensor(out=tmP.rearrange("p g (r a) -> p g r a", a=8),
                                in0=pol4, in1=wvyb, op=ALU.mult)
        tmA = gsp.tile([128, G, N * A], F32)
        nc.gpsimd.tensor_tensor(out=tmA.rearrange("p g (r a) -> p g r a", a=8),
                                in0=act4, in1=wvyb, op=ALU.mult)

        def gtree8(tm, nm):
            v = tm.rearrange("p g (r a) -> p (g r) a", a=8)
            t1 = gsp.tile([128, 64, 4], F32, name=f"t1{nm}")
            nc.gpsimd.tensor_add(t1, v[:, :, 0:4], v[:, :, 4:8])
            t2 = gsp.tile([128, 64, 2], F32, name=f"t2{nm}")
            nc.gpsimd.tensor_add(t2, t1[:, :, 0:2], t1[:, :, 2:4])
            t3 = gsp.tile([128, 64], F32, name=f"t3{nm}")
            nc.gpsimd.tensor_add(t3, t2[:, :, 0:1].rearrange("p i o -> p (i o)"),
                                 t2[:, :, 1:2].rearrange("p i o -> p (i o)"))
            return t3

        P64 = gtree8(tmP, "P")
        A64 = gtree8(tmA, "A")
        Q64 = gsp.tile([128, 64], F32)
        nc.gpsimd.tensor_sub(Q64, A64, P64)

        def gtree16(t64, nm):
            v = t64.rearrange("p (i n) -> p i n", n=16)
            u1 = gsp.tile([128, 4, 8], F32, name=f"u1{nm}")
            nc.gpsimd.tensor_add(u1, v[:, :, 0:8], v[:, :, 8:16])
            u2 = gsp.tile([128, 4, 4], F32, name=f"u2{nm}")
            nc.gpsimd.tensor_add(u2, u1[:, :, 0:4], u1[:, :, 4:8])
            u3 = gsp.tile([128, 4, 2], F32, name=f"u3{nm}")
            nc.gpsimd.tensor_add(u3, u2[:, :, 0:2], u2[:, :, 2:4])
            u4 = gsp.tile([128, 4], F32, name=f"u4{nm}")
            nc.gpsimd.tensor_add(u4, u3[:, :, 0:1].rearrange("p i o -> p (i o)"),
                                 u3[:, :, 1:2].rearrange("p i o -> p (i o)"))
            return u4

        PS4 = gtree16(P64, "P")
        QS4 = gtree16(Q64, "Q")

        m2, t2l = tree_full(2)
        lr1, lr1i = lr_op(1, wc1)
        _dep(lr1i, t2l)
        post(1, wc1, lr1)
        wc2 = chain(2, m2)
        m3, t3l = tree_full(3)
        lr2, lr2i = lr_op(2, wc2)
        _dep(lr2i, t3l)
        post(2, wc2, lr2)
        # all groups: xv = w*(QS-Q)/16 + (c + PS/16); the w/c-independent
        # parts precompute as soon as Q64/QS4/PS4 exist
        qd16 = smp.tile([128, 4], F32)
        nc.gpsimd.tensor_scalar_mul(qd16, QS4, 1.0 / N)
        psd16 = smp.tile([128, 4], F32)
        nc.gpsimd.tensor_scalar_mul(psd16, PS4, 1.0 / N)
        U16 = smp.tile([128, 4, 16], F32)
        u16i = nc.vector.scalar_tensor_tensor(
            out=U16, in0=Q64.rearrange("p (i n) -> p i n", n=16),
            scalar=-1.0 / N,
            in1=qd16.unsqueeze(2).broadcast_to([128, 4, 16]),
            op0=ALU.mult, op1=ALU.add)
        _dep(u16i, t3l)

        def xcombine2(g, wcol, cps_g):
            i = nc.vector.scalar_tensor_tensor(
                out=xwbig[g][:, :, 0:N],
                in0=U16[:, g].unsqueeze(1).broadcast_to([128, 16, 16]),
                scalar=wcol,
                in1=cps_g.unsqueeze(1).broadcast_to([128, 16, 16]),
                op0=ALU.mult, op1=ALU.add)
            nc.sync.dma_start(out=xw_v[g],
                              in_=xwbig[g].rearrange("p d j -> p (d j)"))
            return i

        for g in range(3):
            cps_g = smp.tile([128, 1], F32, name=f"cps_{g}")
            nc.gpsimd.tensor_add(cps_g, wc8[:, 4 + g:5 + g],
                                 psd16[:, g:g + 1])
            xgi = xcombine2(g, wc8[:, g:g + 1], cps_g)
            _dep(xgi, t3l)
        wc3 = chain(3, m3)
        lr3, lr3i = lr_op(3, wc3)
        _dep(lr3i, t3l)
        pw3 = post(3, wc3, lr3)
        cps = smp.tile([128, 1], F32)
        nc.vector.tensor_add(cps, pw3[:, 256:257], psd16[:, 3:4])
        nc.vector.scalar_tensor_tensor(
            out=xwbig[3][:, :, 0:N],
            in0=U16[:, 3].unsqueeze(1).broadcast_to([128, 16, 16]),
            scalar=pw3[:, 0:1],
            in1=cps.unsqueeze(1).broadcast_to([128, 16, 16]),
            op0=ALU.mult, op1=ALU.add)
        # ship group 3 as two d-halves on both rings concurrently
        xwf3 = xwbig[3].rearrange("p d j -> p (d j)")
        nc.sync.dma_start(out=xw_v[3][:, 0:256], in_=xwf3[:, 0:256])
        nc.scalar.dma_start(out=xw_v[3][:, 256:512], in_=xwf3[:, 256:512])

    nc.compile()
    return nc


_NC_CACHE = {}


def _get_nc():
    if "nc" not in _NC_CACHE:
        _NC_CACHE["nc"] = _build()
    return _NC_CACHE["nc"]


def _make_in_maps(inputs):
    obs = np.ascontiguousarray(np.asarray(inputs["obs"], np.float32))
    pol = np.ascontiguousarray(np.asarray(inputs["policies"], np.float32))
    act = np.ascontiguousarray(np.asarray(inputs["actions"], np.float32))
    W1 = np.asarray(inputs["W1"], np.float32)
    b1 = np.asarray(inputs["b1"], np.float32)
    W2 = np.asarray(inputs["W2"], np.float32)
    b2 = np.asarray(inputs["b2"], np.float32)
    Wfc = np.asarray(inputs["Wfc"], np.float32)
    Wattn = np.asarray(inputs["Wattn"], np.float32)
    Wv = np.asarray(inputs["Wv"], np.float32)
    bv = np.asarray(inputs["bv"], np.float32)

    wa = (Wfc @ (Wattn[:DZ] + Wattn[DZ:]))[:, 0]
    wvy = Wv[DP:, 0]
    wv64 = Wv[:DP, 0]

    cst = np.zeros((128, 10), np.float32)
    cst[:, 0:8] = wvy[None, :]
    cst[0:64, 8] = b1
    cst[0, 9] = float(b2 @ wa)
    cst[1, 9] = float(b2 @ wv64 + bv[0])
    cstb = np.zeros((128, 452), np.float32)
    cstb[:, 0:128] = np.eye(128, dtype=np.float32)
    cstb[:, 128:192] = W1 / 16.0
    cstb[0:64, 192] = W2 @ wa
    cstb[0:64, 193] = W2 @ wv64
    cstb[0, 195:451] = 1.0
    cstb[1, 451] = 1.0
    cstb = cstb.astype(ml_dtypes.bfloat16)

    in_maps = []
    for c in range(NCORES):
        in_maps.append({
            "obs": obs[c * RC:(c + 1) * RC],
            "pol": pol[c * RC:(c + 1) * RC],
            "act": act[c * RC:(c + 1) * RC],
            "cst": cst,
            "cstb": cstb,
        })
    return in_maps


TRACE = False
TRACE_KWARGS = {}
LAST_RESULT = None


def kernel(**inputs):
    global LAST_RESULT
    nc = _get_nc()
    in_maps = _make_in_maps(inputs)
    res = run_bass_kernel_spmd(nc, in_maps, core_ids=list(range(NCORES)),
                               trace=TRACE, **TRACE_KWARGS)
    LAST_RESULT = res
    xwf = np.concatenate([r["xw"] for r in res.results], axis=0)
    x = np.ascontiguousarray(xwf[:, 0:N]).reshape(B * N, N, 1)
    w = np.ascontiguousarray(xwf[:, N:2 * N]).reshape(B * N, N, 1)
    return x, w
